# revision 1
# baseline (speedup 1.0000x reference)
"""2-layer GCN (GCNConv -> ReLU -> GCNConv -> log_softmax) on 8 TRN2 NeuronCores.

Strategy (dest-sharded, per the halo-exchange hint):
  - Nodes (and their incident edges, by destination) are partitioned across the
    8 cores: core c owns destination nodes [c*12500, (c+1)*12500).
  - gcn_norm coefficients (degrees / D^-1/2 scaling) are host-side graph
    preprocessing; the per-edge coefficient w' = dinv[src]*w*dinv[dst] is
    folded into one scalar per edge.
  - The halo exchange (gather of remote source features) is materialized on
    the host as a per-core, destination-ordered ELLPACK message stream: this
    turns the device-side work into pure sequential streaming.  (Measured on
    this hardware: every device-side random-access primitive - indirect DMA,
    dma_gather, gpsimd ap_gather - costs 25-200ns per edge, which is 10-100x
    slower than streaming; so the gather is done at input-layout time.)
  - Device kernel 1: stream x-messages, weighted segment-sum over edge slots
    (DVE), transpose (PE), @W1 + b1, ReLU  -> z shard per core.
  - Host: second halo exchange - gather z rows into layer-2 message streams.
  - Device kernel 2: stream z-messages, weighted segment-sum, @W2 + b2,
    log_softmax -> output shard per core.
  - Linear layers are applied AFTER aggregation (A@(X@W1) == (A@X)@W1), so
    all feature arithmetic (the actual FLOPs) happens on device.
"""

import sys

sys.path.insert(0, "/opt/trn_rl_repo")

import numpy as np

from concourse import bass, mybir, bacc
import concourse.tile as tile
from concourse import bass_utils
from concourse.masks import make_identity

N = 100_000
NCORES = 8
DPC = N // NCORES            # 12500 dests per core
P = 128                      # partitions
NWIN = (DPC + P - 1) // P    # 98 windows of 128 dests
DPC_PAD = NWIN * P           # 12544

F_IN = 37
H = 16
C = 2


# ----------------------------------------------------------------------------
# Host-side graph preprocessing (indices / weights only - no feature math)
# ----------------------------------------------------------------------------

def preprocess_graph(edge_index, edge_weight):
    row = np.asarray(edge_index[0]).astype(np.int64)
    col = np.asarray(edge_index[1]).astype(np.int64)
    w = np.asarray(edge_weight).astype(np.float32)

    loop = np.arange(N, dtype=np.int64)
    row = np.concatenate([row, loop])
    col = np.concatenate([col, loop])
    w = np.concatenate([w, np.ones(N, np.float32)])

    deg = np.bincount(col, weights=w.astype(np.float64), minlength=N)
    dinv = np.where(deg > 0, 1.0 / np.sqrt(deg), 0.0).astype(np.float32)
    wn = dinv[row] * w * dinv[col]  # [E+N] f32

    core = col // DPC
    shards = []
    for c in range(NCORES):
        m = core == c
        shards.append((row[m], col[m] - c * DPC, wn[m]))

    # per-core degree-sorted dest permutation (uniform geometry across cores)
    perms, counts_sorted = [], []
    for c in range(NCORES):
        _, ld, _ = shards[c]
        cnt = np.bincount(ld, minlength=DPC)
        order = np.argsort(-cnt, kind="stable")       # rank -> local dest
        permpos = np.empty(DPC, np.int64)
        permpos[order] = np.arange(DPC)               # local dest -> rank
        perms.append((order, permpos))
        cs = np.zeros(DPC_PAD, np.int64)
        cs[: DPC] = cnt[order]
        counts_sorted.append(cs)

    # shared window widths: max over cores of max count within each window
    cnt_all = np.stack(counts_sorted)                 # [8, 12544]
    Lw = cnt_all.reshape(NCORES, NWIN, P).max(axis=(0, 2)).astype(np.int64)
    Lw = np.maximum(Lw, 1)
    off = np.concatenate([[0], np.cumsum(Lw)])
    S = int(off[-1])

    # per-core slot assignment: (128, S) arrays of src node id and w'
    srcpos_all, wn_all = [], []
    for c in range(NCORES):
        src, ld, wnc = shards[c]
        _, permpos = perms[c]
        q = permpos[ld]                                # rank of each edge's dest
        sort = np.argsort(q, kind="stable")
        qs, srcs, wns = q[sort], src[sort], wnc[sort]
        # within-dest slot index
        cnt = np.bincount(qs, minlength=DPC_PAD)
        starts = np.concatenate([[0], np.cumsum(cnt)])[:-1]
        slot = np.arange(len(qs)) - starts[qs]
        wi = qs // P
        colidx = off[wi] + slot
        pi = qs % P
        sp = np.zeros((P, S), np.int64)
        wa = np.zeros((P, S), np.float32)
        sp[pi, colidx] = srcs
        wa[pi, colidx] = wns
        srcpos_all.append(sp)
        wn_all.append(wa)

    return {
        "Lw": Lw, "off": off, "S": S,
        "srcpos": srcpos_all, "wn": wn_all, "perms": perms,
    }


# ----------------------------------------------------------------------------
# Device program: stream messages -> weighted segment-sum -> @W + b -> act
# ----------------------------------------------------------------------------

def build_layer_program(F, OutF, S, Lw, off, last, loop_reps=1):
    """F: message width (37 or 16). OutF: output width (16 or 2).
    last: if True apply log_softmax epilogue, else ReLU."""
    nc = bacc.Bacc("TRN2", target_bir_lowering=False, debug=False,
                   num_devices=NCORES)
    f32 = mybir.dt.float32
    msg_d = nc.dram_tensor("msg", [P, S * F], f32, kind="ExternalInput").ap()
    wn_d = nc.dram_tensor("wn", [P, S], f32, kind="ExternalInput").ap()
    W_d = nc.dram_tensor("W", [F, OutF], f32, kind="ExternalInput").ap()
    b_d = nc.dram_tensor("b", [P, OutF], f32, kind="ExternalInput").ap()
    out_d = nc.dram_tensor("out", [DPC_PAD, OutF], f32, kind="ExternalOutput").ap()
    out_v = out_d.rearrange("(w p) f -> p w f", p=P)

    BATCH = 8  # windows per staged output DMA
    maxL = int(max(Lw))

    with tile.TileContext(nc) as tc:
        with tc.tile_pool(name="const", bufs=1) as cpool, \
             tc.tile_pool(name="sbuf", bufs=3) as pool, \
             tc.tile_pool(name="psum", bufs=2, space="PSUM") as ppool:
            wn_sb = cpool.tile([P, S], f32)
            W_sb = cpool.tile([F, OutF], f32)
            b_sb = cpool.tile([P, OutF], f32)
            ident = cpool.tile([P, P], f32)
            nc.sync.dma_start(out=wn_sb[:], in_=wn_d[:])
            nc.sync.dma_start(out=W_sb[:], in_=W_d[:])
            nc.sync.dma_start(out=b_sb[:], in_=b_d[:])
            make_identity(nc, ident[:])

            def windows():
                stage = None
                for w in range(NWIN):
                    L, o = int(Lw[w]), int(off[w])
                    if w % BATCH == 0:
                        stage = pool.tile([P, BATCH * OutF], f32, tag="stage")
                    msg = pool.tile([P, maxL * F], f32, tag="msg")
                    nc.sync.dma_start(out=msg[:, : L * F],
                                      in_=msg_d[:, o * F:(o + L) * F])
                    m3 = msg[:, : L * F].rearrange("p (s f) -> p s f", f=F)
                    wb = wn_sb[:, o:o + L].unsqueeze(-1).to_broadcast([P, L, F])
                    nc.vector.tensor_tensor(out=m3, in0=m3, in1=wb,
                                            op=mybir.AluOpType.mult)
                    agg = pool.tile([P, F], f32, tag="agg")
                    mr = msg[:, : L * F].rearrange("p (s f) -> p f s", f=F)
                    nc.vector.tensor_reduce(out=agg[:], in_=mr,
                                            axis=mybir.AxisListType.X,
                                            op=mybir.AluOpType.add)
                    # transpose agg [128,F] -> [F,128], then @W -> [128,OutF]
                    aggT_p = ppool.tile([F, P], f32, tag="aggT_p")
                    nc.tensor.transpose(out=aggT_p[:], in_=agg[:], identity=ident[:])
                    aggT = pool.tile([F, P], f32, tag="aggT")
                    nc.scalar.copy(out=aggT[:], in_=aggT_p[:])
                    h_p = ppool.tile([P, OutF], f32, tag="h_p")
                    nc.tensor.matmul(out=h_p[:], lhsT=aggT[:], rhs=W_sb[:],
                                     start=True, stop=True)
                    sl = stage[:, (w % BATCH) * OutF:(w % BATCH + 1) * OutF]
                    if not last:
                        zt = pool.tile([P, OutF], f32, tag="zt")
                        nc.vector.tensor_tensor(out=zt[:], in0=h_p[:], in1=b_sb[:],
                                                op=mybir.AluOpType.add)
                        nc.scalar.activation(out=sl, in_=zt[:],
                                             func=mybir.ActivationFunctionType.Relu)
                    else:
                        ot = pool.tile([P, OutF], f32, tag="ot")
                        nc.vector.tensor_tensor(out=ot[:], in0=h_p[:], in1=b_sb[:],
                                                op=mybir.AluOpType.add)
                        rmax = pool.tile([P, 1], f32, tag="rmax")
                        nc.vector.tensor_reduce(out=rmax[:], in_=ot[:],
                                                axis=mybir.AxisListType.X,
                                                op=mybir.AluOpType.max)
                        xm = pool.tile([P, OutF], f32, tag="xm")
                        nc.vector.tensor_scalar_sub(xm[:], ot[:], rmax[:])
                        ex = pool.tile([P, OutF], f32, tag="ex")
                        se = pool.tile([P, 1], f32, tag="se")
                        nc.scalar.activation(out=ex[:], in_=xm[:],
                                             func=mybir.ActivationFunctionType.Exp,
                                             accum_out=se[:])
                        lse = pool.tile([P, 1], f32, tag="lse")
                        nc.scalar.activation(out=lse[:], in_=se[:],
                                             func=mybir.ActivationFunctionType.Ln)
                        nc.vector.tensor_scalar_sub(sl, xm[:], lse[:])
                    if w % BATCH == BATCH - 1 or w == NWIN - 1:
                        w0 = (w // BATCH) * BATCH
                        nwin = w - w0 + 1
                        nc.scalar.dma_start(
                            out=out_v[:, w0:w0 + nwin, :],
                            in_=stage[:, : nwin * OutF].rearrange(
                                "p (w f) -> p w f", f=OutF))

            if loop_reps == 1:
                windows()
            else:
                with tc.For_i(0, loop_reps, 1):
                    windows()
    nc.compile()
    return nc


# ----------------------------------------------------------------------------
# Full model
# ----------------------------------------------------------------------------

_CACHE = {}


def _get_programs(S, Lw, off, loop_reps=1):
    key = (S, tuple(Lw), loop_reps)
    if key not in _CACHE:
        k1 = build_layer_program(F_IN, H, S, Lw, off, last=False,
                                 loop_reps=loop_reps)
        k2 = build_layer_program(H, C, S, Lw, off, last=True,
                                 loop_reps=loop_reps)
        _CACHE[key] = (k1, k2)
    return _CACHE[key]


def kernel(x, edge_index, edge_weight, W1, b1, W2, b2, _loop_reps=1,
           _return_all=False):
    x = np.asarray(x, dtype=np.float32)
    W1 = np.asarray(W1, np.float32); b1 = np.asarray(b1, np.float32)
    W2 = np.asarray(W2, np.float32); b2 = np.asarray(b2, np.float32)

    g = preprocess_graph(edge_index, edge_weight)
    S, Lw, off = g["S"], g["Lw"], g["off"]
    k1, k2 = _get_programs(S, Lw, off, _loop_reps)

    b1r = np.broadcast_to(b1, (P, H)).copy()
    in1 = []
    for c in range(NCORES):
        msgx = x[g["srcpos"][c].ravel()].reshape(P, S * F_IN)
        in1.append({"msg": msgx, "wn": g["wn"][c], "W": W1, "b": b1r})
    r1 = bass_utils.run_bass_kernel_spmd(k1, in1, core_ids=list(range(NCORES)))
    zshards = [r1.results[c]["out"] for c in range(NCORES)]  # [12544, 16] each

    # host halo exchange for layer 2: map node id -> row in stacked z shards
    posmap = np.empty(N, np.int64)
    for c in range(NCORES):
        _, permpos = g["perms"][c]
        posmap[c * DPC:(c + 1) * DPC] = c * DPC_PAD + permpos
    zfull = np.concatenate(zshards, axis=0)  # [8*12544, 16]

    b2r = np.broadcast_to(b2, (P, C)).copy()
    in2 = []
    for c in range(NCORES):
        msgz = zfull[posmap[g["srcpos"][c].ravel()]].reshape(P, S * H)
        in2.append({"msg": msgz, "wn": g["wn"][c], "W": W2, "b": b2r})
    r2 = bass_utils.run_bass_kernel_spmd(k2, in2, core_ids=list(range(NCORES)))

    out = np.empty((N, C), np.float32)
    for c in range(NCORES):
        order, _ = g["perms"][c]
        shard = r2.results[c]["out"]          # [12544, C], row q = rank q
        out[c * DPC + order] = shard[: DPC]
    if _return_all:
        return out, zshards, g
    return out



# revision 2
# speedup vs baseline: 1.0819x; 1.0819x over previous
"""2-layer GCN (GCNConv -> ReLU -> GCNConv -> log_softmax) on 8 TRN2 NeuronCores.

v3: instruction-count-minimized design.

Measured facts driving it (this hardware, via loop-differenced microbenches):
  - tensor_reduce with R output rows costs ~4-18us PER ROW (it is lowered
    per-row); a 224-row reduce is ~1000x slower than its element count
    suggests.  -> segment-sum is done as a log2(L) tensor_tensor ADD tree
    instead: each level is ONE instruction over a whole chunk.
  - everything pays ~1us/instruction dispatch floor -> few, large ops.

Structure:
  - pass A: T1 = X @ W1 on device (width 37 -> 16), W1 stationary, 25 matmuls.
  - host: halo-gather T1 rows into dest-ordered ELLPACK message stream,
    (w, f, s)-ordered, uniform slot count per ~25-window chunk.
  - pass B: per chunk: 1 DMA + 1 weighted-multiply (DVE, bf16 2x) +
    ~6 tree-adds -> agg; PE transpose (7-window groups) + fused
    bias+ReLU+cast (ACT) + one block-diagonal W2 matmul per group -> t2
    (width 2).
  - host: halo-gather T2 into width-2 message stream.
  - pass C: same aggregation; +b2 and log_softmax evaluated once over all
    windows at the end.
"""

import sys

sys.path.insert(0, "/opt/trn_rl_repo")

import numpy as np
import ml_dtypes

from concourse import bass, mybir, bacc
import concourse.tile as tile
from concourse import bass_utils
from concourse.masks import make_identity

N = 100_000
NCORES = 8
DPC = N // NCORES            # 12500 dests per core
P = 128                      # partitions
NWIN = (DPC + P - 1) // P    # 98 windows of 128 dest ranks
DPC_PAD = NWIN * P           # 12544
CHW = [28, 28, 21, 21]       # windows per chunk (multiples of TG)
CB = np.concatenate([[0], np.cumsum(CHW)])   # chunk window boundaries
NCH = len(CHW)
TG = 7                       # windows per transpose group

F_IN = 37
H = 16
C = 2

BF16 = ml_dtypes.bfloat16


# ----------------------------------------------------------------------------
# Host-side graph preprocessing (indices / weights only - no feature math)
# ----------------------------------------------------------------------------

def preprocess_graph(edge_index, edge_weight):
    row = np.asarray(edge_index[0]).astype(np.int64)
    col = np.asarray(edge_index[1]).astype(np.int64)
    w = np.asarray(edge_weight).astype(np.float32)

    loop = np.arange(N, dtype=np.int64)
    row = np.concatenate([row, loop])
    col = np.concatenate([col, loop])
    w = np.concatenate([w, np.ones(N, np.float32)])

    deg = np.bincount(col, weights=w.astype(np.float64), minlength=N)
    dinv = np.where(deg > 0, 1.0 / np.sqrt(deg), 0.0).astype(np.float32)
    wn = dinv[row] * w * dinv[col]  # [E+N] f32

    core = col // DPC
    shards = []
    for c in range(NCORES):
        m = core == c
        shards.append((row[m], col[m] - c * DPC, wn[m]))

    # per-core degree-sorted dest permutation (uniform geometry across cores)
    perms, counts_sorted = [], []
    for c in range(NCORES):
        _, ld, _ = shards[c]
        cnt = np.bincount(ld, minlength=DPC)
        order = np.argsort(-cnt, kind="stable")       # rank -> local dest
        permpos = np.empty(DPC, np.int64)
        permpos[order] = np.arange(DPC)               # local dest -> rank
        perms.append((order, permpos))
        cs = np.zeros(DPC_PAD, np.int64)
        cs[: DPC] = cnt[order]
        counts_sorted.append(cs)

    # shared per-CHUNK slot widths: max count within chunk, over all cores
    cnt_all = np.stack(counts_sorted)                 # [8, 12544]
    Lc = []
    for i in range(NCH):
        m = cnt_all[:, CB[i] * P: CB[i + 1] * P].max()
        m = max(int(m), 2)
        Lc.append(m + (m % 2))                        # even
    Lc = np.array(Lc, np.int64)
    offw = np.concatenate([[0], np.cumsum(np.array(CHW) * Lc)])
    SW = int(offw[-1])

    # per-core slot assignment: [P, SW] arrays of src node id and w'
    # slot column for (rank q, slot): chunk i, w_in = q//P - CB[i], p = q%P:
    #   offw[i] + w_in*Lc[i] + slot
    srcidx_all, wn_all = [], []
    for c in range(NCORES):
        src, ld, wnc = shards[c]
        _, permpos = perms[c]
        q = permpos[ld]
        sort = np.argsort(q, kind="stable")
        qs, srcs, wns = q[sort], src[sort], wnc[sort]
        cnt = np.bincount(qs, minlength=DPC_PAD)
        starts = np.concatenate([[0], np.cumsum(cnt)])[:-1]
        slot = np.arange(len(qs)) - starts[qs]
        wg = qs // P
        ci = np.searchsorted(CB, wg, side="right") - 1
        w_in = wg - CB[ci]
        pi = qs % P
        colidx = offw[ci] + w_in * Lc[ci] + slot
        sp = np.zeros((P, SW), np.int32)
        wa = np.zeros((P, SW), np.float32)
        sp[pi, colidx] = srcs.astype(np.int32)
        wa[pi, colidx] = wns
        srcidx_all.append(sp)
        wn_all.append(wa)

    return {
        "Lc": Lc, "offw": offw, "SW": SW,
        "srcidx": srcidx_all, "wn": wn_all, "perms": perms,
    }


def build_msgs(T, srcidx, offw, Lc, F):
    """Gather rows of T ([*, F]) into the (f, w, s)-ordered message stream:
    per chunk i the per-partition block is [F, W_i, Lc[i]]."""
    G = T[srcidx]                                     # [P, SW, F]
    parts = []
    for i in range(NCH):
        W, L = CHW[i], int(Lc[i])
        blk = G[:, offw[i]:offw[i + 1], :].reshape(P, W, L, F)
        parts.append(np.ascontiguousarray(
            blk.transpose(0, 3, 1, 2)).reshape(P, -1))
    return np.concatenate(parts, axis=1)              # [P, F*SW]


# ----------------------------------------------------------------------------
# Device programs
# ----------------------------------------------------------------------------

def build_passA(loop_reps=1):
    """T1^T = (X @ W1)^T: W1 stationary, 25 column-chunk matmuls.
    In: xT [37, 12544] bf16, W1 [37, 16] bf16. Out: t1 [16, 12544] bf16."""
    nc = bacc.Bacc("TRN2", target_bir_lowering=False, debug=False,
                   num_devices=NCORES)
    f32 = mybir.dt.float32
    bf = mybir.dt.bfloat16
    xT_d = nc.dram_tensor("xT", [F_IN, DPC_PAD], bf, kind="ExternalInput").ap()
    W1_d = nc.dram_tensor("W1", [F_IN, H], bf, kind="ExternalInput").ap()
    t1_d = nc.dram_tensor("t1", [H, DPC_PAD], bf, kind="ExternalOutput").ap()

    CW = 512  # psum columns per matmul

    with tile.TileContext(nc) as tc:
        with tc.tile_pool(name="const", bufs=1) as cpool, \
             tc.tile_pool(name="psum", bufs=4, space="PSUM") as ppool:
            xT_sb = cpool.tile([F_IN, DPC_PAD], bf)
            W1_sb = cpool.tile([F_IN, H], bf)
            stage = cpool.tile([H, DPC_PAD], bf)
            nc.sync.dma_start(out=xT_sb[:], in_=xT_d[:])
            nc.sync.dma_start(out=W1_sb[:], in_=W1_d[:])

            def body():
                for c0 in range(0, DPC_PAD, CW):
                    n = min(CW, DPC_PAD - c0)
                    ps = ppool.tile([H, CW], f32, tag="ps")
                    nc.tensor.matmul(out=ps[:, :n], lhsT=W1_sb[:],
                                     rhs=xT_sb[:, c0:c0 + n],
                                     start=True, stop=True)
                    nc.scalar.copy(out=stage[:, c0:c0 + n], in_=ps[:, :n])
                nc.scalar.dma_start(out=t1_d[:], in_=stage[:])

            if loop_reps == 1:
                body()
            else:
                with tc.For_i(0, loop_reps, 1):
                    body()
    nc.compile()
    return nc


def _aggregate_chunk(nc, pool, msg, wn_sb, F, W, L, ow, agg_out):
    """Weighted multiply + TT-add tree over slots on the (f, w, s)-ordered
    msg tile; the final tree level writes fp32 agg_out ([P, W*F] view)
    TRANSPOSED into (w, f) order via a strided output AP."""
    E = W * F * L
    flat = msg[:, :E]
    m3 = flat.rearrange("p (f s) -> p f s", f=F)      # [P, F, W*L]
    wb = wn_sb[:, ow: ow + W * L].unsqueeze(1).to_broadcast([P, F, W * L])
    nc.vector.tensor_tensor(out=m3, in0=m3, in1=wb,
                            op=mybir.AluOpType.mult)
    A = W * F
    t3 = flat.rearrange("p (a s) -> p a s", a=A)      # rows a = f*W + w
    k = L
    while k > 2:
        h = (k + 1) // 2
        h += h & 1          # even h keeps 4B alignment (bf16 2x mode)
        nc.vector.tensor_tensor(out=t3[:, :, 0:k - h],
                                in0=t3[:, :, 0:k - h],
                                in1=t3[:, :, h:k],
                                op=mybir.AluOpType.add)
        k = h
    t4 = flat.rearrange("p (f w s) -> p f w s", f=F, w=W)
    ao = agg_out.rearrange("p (w f) -> p f w", f=F).unsqueeze(-1)
    nc.vector.tensor_tensor(out=ao, in0=t4[:, :, :, 0:1],
                            in1=t4[:, :, :, 1:2], op=mybir.AluOpType.add)


def build_passB(Lc, offw, loop_reps=1):
    """Aggregate width-16 messages, +b1, ReLU, @W2 -> t2 (width 2).
    In: msg [P, 16*SW] bf16 (w,f,s-ordered), wn [P, SW] bf16,
        W2d [112, 14] bf16 (block-diagonal), b1v [112, 1] f32.
    Out: t2 [128, 98*2] bf16, rank-ordered (t2[p, w*2+j], rank w*128+p)."""
    SW = int(offw[-1])
    nc = bacc.Bacc("TRN2", target_bir_lowering=False, debug=False,
                   num_devices=NCORES)
    f32 = mybir.dt.float32
    bf = mybir.dt.bfloat16
    msg_d = nc.dram_tensor("msg", [P, H * SW], bf, kind="ExternalInput").ap()
    wn_d = nc.dram_tensor("wn", [P, SW], bf, kind="ExternalInput").ap()
    W2_d = nc.dram_tensor("W2d", [TG * H, TG * C], bf,
                          kind="ExternalInput").ap()
    b1_d = nc.dram_tensor("b1v", [TG * H, 1], f32, kind="ExternalInput").ap()
    t2_d = nc.dram_tensor("t2", [P, NWIN * C], bf, kind="ExternalOutput").ap()

    maxWL = max(CHW[i] * int(Lc[i]) for i in range(NCH))
    maxW = max(CHW)

    with tile.TileContext(nc) as tc:
        with tc.tile_pool(name="const", bufs=1) as cpool, \
             tc.tile_pool(name="sbuf", bufs=2) as pool, \
             tc.tile_pool(name="psum", bufs=2, space="PSUM") as ppool:
            ident = cpool.tile([P, P], f32)
            wn_sb = cpool.tile([P, SW], bf)
            W2_sb = cpool.tile([TG * H, TG * C], bf)
            b1_sb = cpool.tile([TG * H, 1], f32)
            stage = cpool.tile([P, NWIN * C], bf)
            nc.sync.dma_start(out=wn_sb[:], in_=wn_d[:])
            nc.sync.dma_start(out=W2_sb[:], in_=W2_d[:])
            nc.sync.dma_start(out=b1_sb[:], in_=b1_d[:])
            make_identity(nc, ident[:])

            def body():
                for i in range(NCH):
                    W, L = CHW[i], int(Lc[i])
                    ow = int(offw[i])
                    msg = pool.tile([P, H * maxWL], bf, tag="msg")
                    nc.sync.dma_start(out=msg[:, : W * H * L],
                                      in_=msg_d[:, ow * H: (ow + W * L) * H])
                    agg = pool.tile([P, maxW * H], f32, tag="agg")
                    _aggregate_chunk(nc, pool, msg, wn_sb, H, W, L, ow,
                                     agg[:, : W * H])
                    ops_ = ppool.tile([P, maxW * C], f32, tag="ops")
                    for g in range(W // TG):
                        tp = ppool.tile([TG * H, P], f32, tag="tp")
                        nc.tensor.transpose(
                            out=tp[:], in_=agg[:, g * TG * H:(g + 1) * TG * H],
                            identity=ident[:])
                        rT = pool.tile([TG * H, P], bf, tag="rT")
                        nc.scalar.activation(
                            out=rT[:], in_=tp[:],
                            func=mybir.ActivationFunctionType.Relu,
                            bias=b1_sb[:], scale=1.0)
                        nc.tensor.matmul(
                            out=ops_[:, g * TG * C:(g + 1) * TG * C],
                            lhsT=rT[:], rhs=W2_sb[:],
                            start=True, stop=True)
                    nc.scalar.copy(
                        out=stage[:, int(CB[i]) * C:(int(CB[i]) + W) * C],
                        in_=ops_[:, : W * C])
                nc.scalar.dma_start(out=t2_d[:], in_=stage[:])

            if loop_reps == 1:
                body()
            else:
                with tc.For_i(0, loop_reps, 1):
                    body()
    nc.compile()
    return nc


def build_passC(Lc, offw, loop_reps=1):
    """Aggregate width-2 messages, +b2, log_softmax -> out (width 2).
    In: msg [P, 2*SW] bf16, wn [P, SW] bf16, b2r [128, 2] f32.
    Out: out [128, 98*2] f32, rank-ordered."""
    SW = int(offw[-1])
    nc = bacc.Bacc("TRN2", target_bir_lowering=False, debug=False,
                   num_devices=NCORES)
    f32 = mybir.dt.float32
    bf = mybir.dt.bfloat16
    msg_d = nc.dram_tensor("msg", [P, C * SW], bf, kind="ExternalInput").ap()
    wn_d = nc.dram_tensor("wn", [P, SW], bf, kind="ExternalInput").ap()
    b2_d = nc.dram_tensor("b2r", [P, C], f32, kind="ExternalInput").ap()
    out_d = nc.dram_tensor("out", [P, NWIN * C], f32, kind="ExternalOutput").ap()

    maxWL = max(CHW[i] * int(Lc[i]) for i in range(NCH))

    with tile.TileContext(nc) as tc:
        with tc.tile_pool(name="const", bufs=1) as cpool, \
             tc.tile_pool(name="sbuf", bufs=2) as pool:
            wn_sb = cpool.tile([P, SW], bf)
            b2_sb = cpool.tile([P, C], f32)
            aggall = cpool.tile([P, NWIN * C], f32)
            stage = cpool.tile([P, NWIN * C], f32)
            nc.sync.dma_start(out=wn_sb[:], in_=wn_d[:])
            nc.sync.dma_start(out=b2_sb[:], in_=b2_d[:])

            def body():
                for i in range(NCH):
                    W, L = CHW[i], int(Lc[i])
                    ow = int(offw[i])
                    msg = pool.tile([P, C * maxWL], bf, tag="msg")
                    nc.sync.dma_start(out=msg[:, : W * C * L],
                                      in_=msg_d[:, ow * C: (ow + W * L) * C])
                    _aggregate_chunk(nc, pool, msg, wn_sb, C, W, L, ow,
                                     aggall[:, CB[i] * C:(CB[i] + W) * C])
                # epilogue: +b2 then log_softmax over the C=2 classes
                ab = aggall[:].rearrange("p (w f) -> p w f", f=C)
                bb = b2_sb[:].unsqueeze(1).to_broadcast([P, NWIN, C])
                nc.vector.tensor_tensor(out=ab, in0=ab, in1=bb,
                                        op=mybir.AluOpType.add)
                # max over the two classes, via strided views (one TT op)
                rmax = pool.tile([P, NWIN], f32, tag="rmax")
                av = aggall[:].rearrange("p (w f) -> p w f", f=C)
                nc.vector.tensor_tensor(out=rmax[:].unsqueeze(-1),
                                        in0=av[:, :, 0:1], in1=av[:, :, 1:2],
                                        op=mybir.AluOpType.max)
                xm = pool.tile([P, NWIN * C], f32, tag="xm")
                xm3 = xm[:].rearrange("p (w f) -> p w f", f=C)
                rb = rmax[:].unsqueeze(-1).to_broadcast([P, NWIN, C])
                nc.vector.tensor_tensor(out=xm3, in0=ab, in1=rb,
                                        op=mybir.AluOpType.subtract)
                ex = pool.tile([P, NWIN * C], f32, tag="ex")
                nc.scalar.activation(out=ex[:], in_=xm[:],
                                     func=mybir.ActivationFunctionType.Exp)
                se = pool.tile([P, NWIN], f32, tag="se")
                ev = ex[:].rearrange("p (w f) -> p w f", f=C)
                nc.vector.tensor_tensor(out=se[:].unsqueeze(-1),
                                        in0=ev[:, :, 0:1], in1=ev[:, :, 1:2],
                                        op=mybir.AluOpType.add)
                lse = pool.tile([P, NWIN], f32, tag="lse")
                nc.scalar.activation(out=lse[:], in_=se[:],
                                     func=mybir.ActivationFunctionType.Ln)
                lb = lse[:].unsqueeze(-1).to_broadcast([P, NWIN, C])
                st3 = stage[:].rearrange("p (w f) -> p w f", f=C)
                nc.vector.tensor_tensor(out=st3, in0=xm3, in1=lb,
                                        op=mybir.AluOpType.subtract)
                nc.scalar.dma_start(out=out_d[:], in_=stage[:])

            if loop_reps == 1:
                body()
            else:
                with tc.For_i(0, loop_reps, 1):
                    body()
    nc.compile()
    return nc


# ----------------------------------------------------------------------------
# Full model
# ----------------------------------------------------------------------------

_CACHE = {}


def _get_programs(Lc, offw, loop_reps=1):
    key = (tuple(int(l) for l in Lc), loop_reps)
    if key not in _CACHE:
        _CACHE[key] = (build_passA(loop_reps),
                       build_passB(Lc, offw, loop_reps),
                       build_passC(Lc, offw, loop_reps))
    return _CACHE[key]


def _unstack(dev_out, F):
    """[P, NWIN*F] device layout -> [DPC_PAD, F] (row i = w*128+p)."""
    return np.ascontiguousarray(
        dev_out.reshape(P, NWIN, F).transpose(1, 0, 2)).reshape(DPC_PAD, F)


def kernel(x, edge_index, edge_weight, W1, b1, W2, b2, _return_all=False):
    x = np.asarray(x, dtype=np.float32)
    W1 = np.asarray(W1, np.float32); b1 = np.asarray(b1, np.float32)
    W2 = np.asarray(W2, np.float32); b2 = np.asarray(b2, np.float32)

    g = preprocess_graph(edge_index, edge_weight)
    Lc, offw = g["Lc"], g["offw"]
    kA, kB, kC = _get_programs(Lc, offw)

    cores = list(range(NCORES))

    # ---- pass A: T1 = X @ W1 ------------------------------------------------
    W1b = W1.astype(BF16)
    inA = []
    for c in range(NCORES):
        xs = np.zeros((DPC_PAD, F_IN), np.float32)
        xs[:DPC] = x[c * DPC:(c + 1) * DPC]
        inA.append({"xT": np.ascontiguousarray(xs.T).astype(BF16), "W1": W1b})
    rA = bass_utils.run_bass_kernel_spmd(kA, inA, core_ids=cores)
    T1 = np.empty((N, H), BF16)
    for c in range(NCORES):
        T1[c * DPC:(c + 1) * DPC] = \
            np.ascontiguousarray(rA.results[c]["t1"].T)[:DPC]

    # ---- pass B: aggregate T1-messages, +b1, ReLU, @W2 ----------------------
    W2d = np.zeros((TG * H, TG * C), np.float32)      # block-diagonal W2
    for k in range(TG):
        W2d[k * H:(k + 1) * H, k * C:(k + 1) * C] = W2
    W2d = W2d.astype(BF16)
    b1v = np.tile(b1, TG).reshape(TG * H, 1).astype(np.float32)
    inB = []
    for c in range(NCORES):
        inB.append({"msg": build_msgs(T1, g["srcidx"][c], offw, Lc, H),
                    "wn": g["wn"][c].astype(BF16),
                    "W2d": W2d, "b1v": b1v})
    rB = bass_utils.run_bass_kernel_spmd(kB, inB, core_ids=cores)
    T2 = np.empty((N, C), BF16)
    for c in range(NCORES):
        order, _ = g["perms"][c]
        t2r = _unstack(rB.results[c]["t2"], C)        # rank-ordered
        T2[c * DPC + order] = t2r[:DPC]

    # ---- pass C: aggregate T2-messages, +b2, log_softmax --------------------
    b2r = np.broadcast_to(b2, (P, C)).astype(np.float32).copy()
    inC = []
    for c in range(NCORES):
        inC.append({"msg": build_msgs(T2, g["srcidx"][c], offw, Lc, C),
                    "wn": g["wn"][c].astype(BF16), "b2r": b2r})
    rC = bass_utils.run_bass_kernel_spmd(kC, inC, core_ids=cores)

    out = np.empty((N, C), np.float32)
    for c in range(NCORES):
        order, _ = g["perms"][c]
        orank = _unstack(rC.results[c]["out"], C)
        out[c * DPC + order] = orank[:DPC]
    if _return_all:
        return out, {"g": g, "inA": inA, "inB": inB, "inC": inC}
    return out


# revision 3
# speedup vs baseline: 1.1926x; 1.1023x over previous
"""2-layer GCN (GCNConv -> ReLU -> GCNConv -> log_softmax) on 8 TRN2 NeuronCores.

v3: instruction-count-minimized design.

Measured facts driving it (this hardware, via loop-differenced microbenches):
  - tensor_reduce with R output rows costs ~4-18us PER ROW (it is lowered
    per-row); a 224-row reduce is ~1000x slower than its element count
    suggests.  -> segment-sum is done as a log2(L) tensor_tensor ADD tree
    instead: each level is ONE instruction over a whole chunk.
  - everything pays ~1us/instruction dispatch floor -> few, large ops.

Structure:
  - pass A: T1 = X @ W1 on device (width 37 -> 16), W1 stationary, 25 matmuls.
  - host: halo-gather T1 rows into dest-ordered ELLPACK message stream,
    (w, f, s)-ordered, uniform slot count per ~25-window chunk.
  - pass B: per chunk: 1 DMA + 1 weighted-multiply (DVE, bf16 2x) +
    ~6 tree-adds -> agg; PE transpose (7-window groups) + fused
    bias+ReLU+cast (ACT) + one block-diagonal W2 matmul per group -> t2
    (width 2).
  - host: halo-gather T2 into width-2 message stream.
  - pass C: same aggregation; +b2 and log_softmax evaluated once over all
    windows at the end.
"""

import sys

sys.path.insert(0, "/opt/trn_rl_repo")

import numpy as np
import ml_dtypes

from concourse import bass, mybir, bacc
import concourse.tile as tile
from concourse import bass_utils
from concourse.masks import make_identity

N = 100_000
NCORES = 8
DPC = N // NCORES            # 12500 dests per core
P = 128                      # partitions
NWIN = (DPC + P - 1) // P    # 98 windows of 128 dest ranks
DPC_PAD = NWIN * P           # 12544
CHW = [28, 28, 21, 21]       # windows per chunk (multiples of TG)
CB = np.concatenate([[0], np.cumsum(CHW)])   # chunk window boundaries
NCH = len(CHW)
TG = 7                       # windows per transpose group

F_IN = 37
H = 16
C = 2

BF16 = ml_dtypes.bfloat16


# ----------------------------------------------------------------------------
# Host-side graph preprocessing (indices / weights only - no feature math)
# ----------------------------------------------------------------------------

def preprocess_graph(edge_index, edge_weight):
    row = np.asarray(edge_index[0]).astype(np.int64)
    col = np.asarray(edge_index[1]).astype(np.int64)
    w = np.asarray(edge_weight).astype(np.float32)

    loop = np.arange(N, dtype=np.int64)
    row = np.concatenate([row, loop])
    col = np.concatenate([col, loop])
    w = np.concatenate([w, np.ones(N, np.float32)])

    deg = np.bincount(col, weights=w.astype(np.float64), minlength=N)
    dinv = np.where(deg > 0, 1.0 / np.sqrt(deg), 0.0).astype(np.float32)
    wn = dinv[row] * w * dinv[col]  # [E+N] f32

    core = col // DPC
    shards = []
    for c in range(NCORES):
        m = core == c
        shards.append((row[m], col[m] - c * DPC, wn[m]))

    # per-core degree-sorted dest permutation (uniform geometry across cores)
    perms, counts_sorted = [], []
    for c in range(NCORES):
        _, ld, _ = shards[c]
        cnt = np.bincount(ld, minlength=DPC)
        order = np.argsort(-cnt, kind="stable")       # rank -> local dest
        permpos = np.empty(DPC, np.int64)
        permpos[order] = np.arange(DPC)               # local dest -> rank
        perms.append((order, permpos))
        cs = np.zeros(DPC_PAD, np.int64)
        cs[: DPC] = cnt[order]
        counts_sorted.append(cs)

    # shared per-CHUNK slot widths: max count within chunk, over all cores
    cnt_all = np.stack(counts_sorted)                 # [8, 12544]
    Lc = []
    for i in range(NCH):
        m = cnt_all[:, CB[i] * P: CB[i + 1] * P].max()
        m = max(int(m), 2)
        Lc.append(m + (m % 2))                        # even
    Lc = np.array(Lc, np.int64)
    offw = np.concatenate([[0], np.cumsum(np.array(CHW) * Lc)])
    SW = int(offw[-1])

    # per-core slot assignment: [P, SW] arrays of src node id and w'
    # slot column for (rank q, slot): chunk i, w_in = q//P - CB[i], p = q%P:
    #   offw[i] + w_in*Lc[i] + slot
    srcidx_all, wn_all = [], []
    for c in range(NCORES):
        src, ld, wnc = shards[c]
        _, permpos = perms[c]
        q = permpos[ld]
        sort = np.argsort(q, kind="stable")
        qs, srcs, wns = q[sort], src[sort], wnc[sort]
        cnt = np.bincount(qs, minlength=DPC_PAD)
        starts = np.concatenate([[0], np.cumsum(cnt)])[:-1]
        slot = np.arange(len(qs)) - starts[qs]
        wg = qs // P
        ci = np.searchsorted(CB, wg, side="right") - 1
        w_in = wg - CB[ci]
        pi = qs % P
        # (s, w)-ordered slot columns within each chunk (slot-major)
        colidx = offw[ci] + slot * np.array(CHW)[ci] + w_in
        sp = np.zeros((P, SW), np.int32)
        wa = np.zeros((P, SW), np.float32)
        sp[pi, colidx] = srcs.astype(np.int32)
        wa[pi, colidx] = wns
        srcidx_all.append(sp)
        wn_all.append(wa)

    return {
        "Lc": Lc, "offw": offw, "SW": SW,
        "srcidx": srcidx_all, "wn": wn_all, "perms": perms,
    }


def build_msgs(T, srcidx, offw, Lc, F):
    """Gather rows of T ([*, F]) into the (s, w, f)-ordered message stream:
    per chunk i the per-partition block is [Lc[i], W_i, F] (slot-major).
    srcidx columns are already (s, w)-ordered, so no transpose needed."""
    G = T[srcidx]                                     # [P, SW, F]
    parts = []
    for i in range(NCH):
        parts.append(np.ascontiguousarray(
            G[:, offw[i]:offw[i + 1], :]).reshape(P, -1))
    return np.concatenate(parts, axis=1)              # [P, F*SW]


# ----------------------------------------------------------------------------
# Device programs
# ----------------------------------------------------------------------------

def build_passA(loop_reps=1):
    """T1^T = (X @ W1)^T: W1 stationary, 25 column-chunk matmuls.
    In: xT [37, 12544] bf16, W1 [37, 16] bf16. Out: t1 [16, 12544] bf16."""
    nc = bacc.Bacc("TRN2", target_bir_lowering=False, debug=False,
                   num_devices=NCORES)
    f32 = mybir.dt.float32
    bf = mybir.dt.bfloat16
    xT_d = nc.dram_tensor("xT", [F_IN, DPC_PAD], bf, kind="ExternalInput").ap()
    W1_d = nc.dram_tensor("W1", [F_IN, H], bf, kind="ExternalInput").ap()
    t1_d = nc.dram_tensor("t1", [H, DPC_PAD], bf, kind="ExternalOutput").ap()

    CW = 512  # psum columns per matmul

    with tile.TileContext(nc) as tc:
        with tc.tile_pool(name="const", bufs=1) as cpool, \
             tc.tile_pool(name="psum", bufs=4, space="PSUM") as ppool:
            xT_sb = cpool.tile([F_IN, DPC_PAD], bf)
            W1_sb = cpool.tile([F_IN, H], bf)
            stage = cpool.tile([H, DPC_PAD], bf)
            nc.sync.dma_start(out=xT_sb[:], in_=xT_d[:])
            nc.sync.dma_start(out=W1_sb[:], in_=W1_d[:])

            def body():
                for c0 in range(0, DPC_PAD, CW):
                    n = min(CW, DPC_PAD - c0)
                    ps = ppool.tile([H, CW], f32, tag="ps")
                    nc.tensor.matmul(out=ps[:, :n], lhsT=W1_sb[:],
                                     rhs=xT_sb[:, c0:c0 + n],
                                     start=True, stop=True)
                    nc.scalar.copy(out=stage[:, c0:c0 + n], in_=ps[:, :n])
                nc.scalar.dma_start(out=t1_d[:], in_=stage[:])

            if loop_reps == 1:
                body()
            else:
                with tc.For_i(0, loop_reps, 1):
                    body()
    nc.compile()
    return nc


def _aggregate_chunk(nc, pool, msg, wn_sb, F, W, L, ow, agg_out,
                     mult_engine=None):
    """Weighted multiply + TT-add tree over slots on the (s, w, f)-ordered
    (slot-major) msg tile. Every tree level is a pure 2D contiguous add
    (slabs of W*F), and the s=0 slab is already (w, f)-ordered, so the
    final level writes fp32 agg_out directly.

    mult_engine: engine for the weighted multiply. The innermost-stride-0
    broadcast AP runs ~36 cyc/elem on the DVE but near line-rate on
    GPSIMD's HW read-pattern path, and GPSIMD overlaps with the DVE tree
    of the previous chunk."""
    E = W * F * L
    WF = W * F
    flat = msg[:, :E]
    m3 = flat.rearrange("p (a f) -> p a f", f=F)      # [P, L*W, F]
    wb = wn_sb[:, ow: ow + W * L].unsqueeze(-1).to_broadcast([P, L * W, F])
    (mult_engine or nc.vector).tensor_tensor(out=m3, in0=m3, in1=wb,
                                             op=mybir.AluOpType.mult)
    k = L
    while k > 2:
        h = (k + 1) // 2
        nc.vector.tensor_tensor(out=flat[:, 0:(k - h) * WF],
                                in0=flat[:, 0:(k - h) * WF],
                                in1=flat[:, h * WF:k * WF],
                                op=mybir.AluOpType.add)
        k = h
    nc.vector.tensor_tensor(out=agg_out, in0=flat[:, 0:WF],
                            in1=flat[:, WF:2 * WF], op=mybir.AluOpType.add)


def build_passB(Lc, offw, loop_reps=1):
    """Aggregate width-16 messages, +b1, ReLU, @W2 -> t2 (width 2).
    In: msg [P, 16*SW] bf16 (w,f,s-ordered), wn [P, SW] bf16,
        W2d [112, 14] bf16 (block-diagonal), b1v [112, 1] f32.
    Out: t2 [128, 98*2] bf16, rank-ordered (t2[p, w*2+j], rank w*128+p)."""
    SW = int(offw[-1])
    nc = bacc.Bacc("TRN2", target_bir_lowering=False, debug=False,
                   num_devices=NCORES)
    f32 = mybir.dt.float32
    bf = mybir.dt.bfloat16
    msg_d = nc.dram_tensor("msg", [P, H * SW], bf, kind="ExternalInput").ap()
    wn_d = nc.dram_tensor("wn", [P, SW], bf, kind="ExternalInput").ap()
    W2_d = nc.dram_tensor("W2d", [TG * H, TG * C], bf,
                          kind="ExternalInput").ap()
    b1_d = nc.dram_tensor("b1v", [TG * H, 1], f32, kind="ExternalInput").ap()
    t2_d = nc.dram_tensor("t2", [P, NWIN * C], bf, kind="ExternalOutput").ap()

    maxWL = max(CHW[i] * int(Lc[i]) for i in range(NCH))
    maxW = max(CHW)

    with tile.TileContext(nc) as tc:
        with tc.tile_pool(name="const", bufs=1) as cpool, \
             tc.tile_pool(name="sbuf", bufs=2) as pool, \
             tc.tile_pool(name="psum", bufs=2, space="PSUM") as ppool:
            ident = cpool.tile([P, P], f32)
            wn_sb = cpool.tile([P, SW], bf)
            W2_sb = cpool.tile([TG * H, TG * C], bf)
            b1_sb = cpool.tile([TG * H, 1], f32)
            stage = cpool.tile([P, NWIN * C], bf)
            nc.sync.dma_start(out=wn_sb[:], in_=wn_d[:])
            nc.sync.dma_start(out=W2_sb[:], in_=W2_d[:])
            nc.sync.dma_start(out=b1_sb[:], in_=b1_d[:])
            make_identity(nc, ident[:])

            def body():
                for i in range(NCH):
                    W, L = CHW[i], int(Lc[i])
                    ow = int(offw[i])
                    msg = pool.tile([P, H * maxWL], bf, tag="msg")
                    nc.sync.dma_start(out=msg[:, : W * H * L],
                                      in_=msg_d[:, ow * H: (ow + W * L) * H])
                    agg = pool.tile([P, maxW * H], f32, tag="agg")
                    _aggregate_chunk(nc, pool, msg, wn_sb, H, W, L, ow,
                                     agg[:, : W * H], mult_engine=nc.gpsimd)
                    ops_ = ppool.tile([P, maxW * C], f32, tag="ops")
                    for g in range(W // TG):
                        tp = ppool.tile([TG * H, P], f32, tag="tp")
                        nc.tensor.transpose(
                            out=tp[:], in_=agg[:, g * TG * H:(g + 1) * TG * H],
                            identity=ident[:])
                        rT = pool.tile([TG * H, P], bf, tag="rT")
                        nc.scalar.activation(
                            out=rT[:], in_=tp[:],
                            func=mybir.ActivationFunctionType.Relu,
                            bias=b1_sb[:], scale=1.0)
                        nc.tensor.matmul(
                            out=ops_[:, g * TG * C:(g + 1) * TG * C],
                            lhsT=rT[:], rhs=W2_sb[:],
                            start=True, stop=True)
                    nc.scalar.copy(
                        out=stage[:, int(CB[i]) * C:(int(CB[i]) + W) * C],
                        in_=ops_[:, : W * C])
                nc.scalar.dma_start(out=t2_d[:], in_=stage[:])

            if loop_reps == 1:
                body()
            else:
                with tc.For_i(0, loop_reps, 1):
                    body()
    nc.compile()
    return nc


def build_passC(Lc, offw, loop_reps=1):
    """Aggregate width-2 messages, +b2, log_softmax -> out (width 2).
    In: msg [P, 2*SW] bf16, wn [P, SW] bf16, b2r [128, 2] f32.
    Out: out [128, 98*2] f32, rank-ordered."""
    SW = int(offw[-1])
    nc = bacc.Bacc("TRN2", target_bir_lowering=False, debug=False,
                   num_devices=NCORES)
    f32 = mybir.dt.float32
    bf = mybir.dt.bfloat16
    msg_d = nc.dram_tensor("msg", [P, C * SW], bf, kind="ExternalInput").ap()
    wn_d = nc.dram_tensor("wn", [P, SW], bf, kind="ExternalInput").ap()
    b2_d = nc.dram_tensor("b2r", [P, C], f32, kind="ExternalInput").ap()
    out_d = nc.dram_tensor("out", [P, NWIN * C], f32, kind="ExternalOutput").ap()

    maxWL = max(CHW[i] * int(Lc[i]) for i in range(NCH))

    with tile.TileContext(nc) as tc:
        with tc.tile_pool(name="const", bufs=1) as cpool, \
             tc.tile_pool(name="sbuf", bufs=2) as pool:
            wn_sb = cpool.tile([P, SW], bf)
            b2_sb = cpool.tile([P, C], f32)
            aggall = cpool.tile([P, NWIN * C], f32)
            stage = cpool.tile([P, NWIN * C], f32)
            nc.sync.dma_start(out=wn_sb[:], in_=wn_d[:])
            nc.sync.dma_start(out=b2_sb[:], in_=b2_d[:])

            def body():
                for i in range(NCH):
                    W, L = CHW[i], int(Lc[i])
                    ow = int(offw[i])
                    msg = pool.tile([P, C * maxWL], bf, tag="msg")
                    nc.sync.dma_start(out=msg[:, : W * C * L],
                                      in_=msg_d[:, ow * C: (ow + W * L) * C])
                    _aggregate_chunk(nc, pool, msg, wn_sb, C, W, L, ow,
                                     aggall[:, CB[i] * C:(CB[i] + W) * C])
                # epilogue: +b2 then log_softmax over the C=2 classes
                ab = aggall[:].rearrange("p (w f) -> p w f", f=C)
                bb = b2_sb[:].unsqueeze(1).to_broadcast([P, NWIN, C])
                nc.vector.tensor_tensor(out=ab, in0=ab, in1=bb,
                                        op=mybir.AluOpType.add)
                # max over the two classes, via strided views (one TT op)
                rmax = pool.tile([P, NWIN], f32, tag="rmax")
                av = aggall[:].rearrange("p (w f) -> p w f", f=C)
                nc.vector.tensor_tensor(out=rmax[:].unsqueeze(-1),
                                        in0=av[:, :, 0:1], in1=av[:, :, 1:2],
                                        op=mybir.AluOpType.max)
                xm = pool.tile([P, NWIN * C], f32, tag="xm")
                xm3 = xm[:].rearrange("p (w f) -> p w f", f=C)
                rb = rmax[:].unsqueeze(-1).to_broadcast([P, NWIN, C])
                nc.vector.tensor_tensor(out=xm3, in0=ab, in1=rb,
                                        op=mybir.AluOpType.subtract)
                ex = pool.tile([P, NWIN * C], f32, tag="ex")
                nc.scalar.activation(out=ex[:], in_=xm[:],
                                     func=mybir.ActivationFunctionType.Exp)
                se = pool.tile([P, NWIN], f32, tag="se")
                ev = ex[:].rearrange("p (w f) -> p w f", f=C)
                nc.vector.tensor_tensor(out=se[:].unsqueeze(-1),
                                        in0=ev[:, :, 0:1], in1=ev[:, :, 1:2],
                                        op=mybir.AluOpType.add)
                lse = pool.tile([P, NWIN], f32, tag="lse")
                nc.scalar.activation(out=lse[:], in_=se[:],
                                     func=mybir.ActivationFunctionType.Ln)
                lb = lse[:].unsqueeze(-1).to_broadcast([P, NWIN, C])
                st3 = stage[:].rearrange("p (w f) -> p w f", f=C)
                nc.vector.tensor_tensor(out=st3, in0=xm3, in1=lb,
                                        op=mybir.AluOpType.subtract)
                nc.scalar.dma_start(out=out_d[:], in_=stage[:])

            if loop_reps == 1:
                body()
            else:
                with tc.For_i(0, loop_reps, 1):
                    body()
    nc.compile()
    return nc


# ----------------------------------------------------------------------------
# Full model
# ----------------------------------------------------------------------------

_CACHE = {}


def _get_programs(Lc, offw, loop_reps=1):
    key = (tuple(int(l) for l in Lc), loop_reps)
    if key not in _CACHE:
        _CACHE[key] = (build_passA(loop_reps),
                       build_passB(Lc, offw, loop_reps),
                       build_passC(Lc, offw, loop_reps))
    return _CACHE[key]


def _unstack(dev_out, F):
    """[P, NWIN*F] device layout -> [DPC_PAD, F] (row i = w*128+p)."""
    return np.ascontiguousarray(
        dev_out.reshape(P, NWIN, F).transpose(1, 0, 2)).reshape(DPC_PAD, F)


def kernel(x, edge_index, edge_weight, W1, b1, W2, b2, _return_all=False):
    x = np.asarray(x, dtype=np.float32)
    W1 = np.asarray(W1, np.float32); b1 = np.asarray(b1, np.float32)
    W2 = np.asarray(W2, np.float32); b2 = np.asarray(b2, np.float32)

    g = preprocess_graph(edge_index, edge_weight)
    Lc, offw = g["Lc"], g["offw"]
    kA, kB, kC = _get_programs(Lc, offw)

    cores = list(range(NCORES))

    # ---- pass A: T1 = X @ W1 ------------------------------------------------
    W1b = W1.astype(BF16)
    inA = []
    for c in range(NCORES):
        xs = np.zeros((DPC_PAD, F_IN), np.float32)
        xs[:DPC] = x[c * DPC:(c + 1) * DPC]
        inA.append({"xT": np.ascontiguousarray(xs.T).astype(BF16), "W1": W1b})
    rA = bass_utils.run_bass_kernel_spmd(kA, inA, core_ids=cores)
    T1 = np.empty((N, H), BF16)
    for c in range(NCORES):
        T1[c * DPC:(c + 1) * DPC] = \
            np.ascontiguousarray(rA.results[c]["t1"].T)[:DPC]

    # ---- pass B: aggregate T1-messages, +b1, ReLU, @W2 ----------------------
    W2d = np.zeros((TG * H, TG * C), np.float32)      # block-diagonal W2
    for k in range(TG):
        W2d[k * H:(k + 1) * H, k * C:(k + 1) * C] = W2
    W2d = W2d.astype(BF16)
    b1v = np.tile(b1, TG).reshape(TG * H, 1).astype(np.float32)
    inB = []
    for c in range(NCORES):
        inB.append({"msg": build_msgs(T1, g["srcidx"][c], offw, Lc, H),
                    "wn": g["wn"][c].astype(BF16),
                    "W2d": W2d, "b1v": b1v})
    rB = bass_utils.run_bass_kernel_spmd(kB, inB, core_ids=cores)
    T2 = np.empty((N, C), BF16)
    for c in range(NCORES):
        order, _ = g["perms"][c]
        t2r = _unstack(rB.results[c]["t2"], C)        # rank-ordered
        T2[c * DPC + order] = t2r[:DPC]

    # ---- pass C: aggregate T2-messages, +b2, log_softmax --------------------
    b2r = np.broadcast_to(b2, (P, C)).astype(np.float32).copy()
    inC = []
    for c in range(NCORES):
        inC.append({"msg": build_msgs(T2, g["srcidx"][c], offw, Lc, C),
                    "wn": g["wn"][c].astype(BF16), "b2r": b2r})
    rC = bass_utils.run_bass_kernel_spmd(kC, inC, core_ids=cores)

    out = np.empty((N, C), np.float32)
    for c in range(NCORES):
        order, _ = g["perms"][c]
        orank = _unstack(rC.results[c]["out"], C)
        out[c * DPC + order] = orank[:DPC]
    if _return_all:
        return out, {"g": g, "inA": inA, "inB": inB, "inC": inC}
    return out
